# revision 40
# baseline (speedup 1.0000x reference)
"""ConvolutionalAttention (training branch) for Trainium2, 8 NeuronCores.

The module computes, per sample b:
    out[:, :32]  = conv13x13(x1, lk_filter) + depthwise3x3(x1, dyn_k[b])
    out[:, 32:]  = x2            (pass-through)
where dyn_k[b] comes from a tiny MLP (pool -> 1x1 -> GELU -> 1x1) on x1.

Key transformation: conv is linear in the filter, so the per-sample dynamic
depthwise 3x3 kernel is folded host-side into the center of a per-sample
13x13 dense filter.  The device then runs ONE dense 32->32 13x13 conv per
sample.  Data-parallel over batch: 2 samples per core.

Device mapping (per core, per sample):
  - conv as matmul with K = 128 = (4 row-shift replicas g) x (32 in-ch),
    M = 128 = (4 output rows dy) x (32 out-ch), rhs free dim N = 384 =
    two 4-row "quads" (8 output rows) read via an overlapped access pattern
    (pad columns excluded from the moving dim).
  - 52 weight blocks (4 ky'-chunks x 13 kx shifts) accumulate in PSUM.
  - bf16 inputs/weights (quantized host-side), fp32 PSUM accumulate:
    full PE rate, halved DMA/LDWEIGHTS cost, ~2.5e-3 relative error.
  - input DMAs on sync+scalar queues (contiguous per-partition runs),
    outputs on gpsimd; startup stripes band 0 + the first weight chunk
    across all three queues.
"""

import json

import numpy as np

import concourse.bass as bass
import concourse.mybir as mybir
import concourse.tile as tile
from concourse.bass_utils import run_bass_kernel_spmd

# ---------------------------------------------------------------------------
# Problem constants (hardcoded; kernel.py must be self-contained)
B, C, H, W = 16, 64, 192, 192
PD, SK, LK = 32, 3, 13
PAD = LK // 2                      # 6
NCORES = 8
BLOC = B // NCORES                 # 2 samples per core
PADW = W + 2 * PAD                 # 204
PADH = H + 2 * PAD                 # 204
NJ, G, DY = 4, 4, 4                # ky' chunks, row-shift replicas, rows/quad
NKX = LK                           # 13 kx shifts
NBLK = NJ * NKX                    # 52 weight blocks per sample
BANDS = 6                          # 32 output rows per band
PAIRS = 4                          # quad-pairs per band (8 rows each)
SROWS = 41                         # X4 rows needed per band
NFREE = 2 * W                      # 384 matmul moving free dim (pad cols cut)
F32 = mybir.dt.float32
BF16 = mybir.dt.bfloat16

# ---------------------------------------------------------------------------
# BIR post-processing:
# 1) elide redundant Ldweights: bf16 matmuls are emitted as explicit
#    Ldweights+Matmult pairs; each weight block feeds several consecutive
#    matmuls, so all but the first load of a block are dropped (waits carried
#    by a dropped load are preserved on a NoOp).
# 2) the walrus_driver in this container rejects instructions with more than
#    one sync-wait command; move excess waits onto single-wait NoOps inserted
#    right before the offending instruction (same engine => executes first,
#    semantics preserved).
_orig_to_json_bytes = bass.Bass.to_json_bytes

_LDW_TRANSPARENT = {"Matmult", "NoOp", "EventSemaphore"}


def _regroup_matmuls(m):
    """The tile scheduler interleaves matmuls of different weight blocks,
    defeating Ldweights reuse.  Regroup: within wait-free runs of PE
    Ldweights/Matmult instructions, reorder matmuls so that all uses of one
    weight block are consecutive (group order = first occurrence).  Safe
    because (a) matmuls on the same PSUM bank are never reordered relative
    to each other (every bank sees weight blocks in the same program
    order), (b) semaphore updates stay attached to their instruction and
    waiters use total counts, (c) any instruction carrying a sync wait acts
    as a barrier that nothing crosses."""
    import json as _json

    def flush(seg, out):
        if len(seg) <= 2:
            out.extend(seg)
            return
        groups = {}
        order = []
        pending = []  # loads waiting for the first matmul of their block
                      # (the scheduler keeps up to two in flight)
        for inst in seg:
            if inst["opcode"] == "Ldweights":
                pending.append(inst)
            else:  # Matmult
                key = _json.dumps(inst["ins"][1], sort_keys=True)
                if key in groups:
                    # reuse of an earlier block: its own reload (if any) is
                    # redundant once the uses are grouped; unrelated pending
                    # loads (prefetches for later blocks) stay pending
                    ridx = next((i for i, l in enumerate(pending)
                                 if _json.dumps(l["ins"][0],
                                                sort_keys=True) == key),
                                None)
                    if ridx is not None:
                        pending.pop(ridx)
                    groups[key].append(inst)
                    continue
                lidx = next((i for i, l in enumerate(pending)
                             if _json.dumps(l["ins"][0],
                                            sort_keys=True) == key), None)
                if lidx is not None:
                    groups[key] = [pending.pop(lidx), inst]
                else:
                    # block loaded before this segment (its load carried a
                    # wait and acts as the segment barrier); group must
                    # stay first, which first-occurrence order guarantees
                    groups[key] = [inst]
                order.append(key)
        for key in order:
            out.extend(groups[key])
        # unconsumed prefetches feed matmuls beyond the next barrier; they
        # must stay last so they cannot clobber any group's weights early
        out.extend(pending)

    for f in m.get("functions", []):
        for blk in f.get("blocks", []):
            out = []
            seg = []
            for inst in blk.get("instructions", []):
                if inst.get("engine") != "PE":
                    out.append(inst)
                    continue
                waits = ((inst.get("sync_info") or {}).get("on_wait")) or []
                if inst["opcode"] in ("Ldweights", "Matmult") and not waits:
                    seg.append(inst)
                    continue
                flush(seg, out)
                seg = []
                out.append(inst)
            flush(seg, out)
            blk["instructions"] = out
    return m


def _elide_redundant_ldweights(m):
    import json as _json
    for f in m.get("functions", []):
        for blk in f.get("blocks", []):
            out = []
            prev_key = None
            for inst in blk.get("instructions", []):
                op = inst.get("opcode")
                if inst.get("engine") != "PE":
                    # other engines cannot touch the PE weight registers
                    out.append(inst)
                    continue
                if op == "Ldweights":
                    key = _json.dumps(
                        [inst.get("ins"), inst.get("tile_size"),
                         inst.get("tile_position"), inst.get("perf_mode"),
                         inst.get("is_transpose")], sort_keys=True)
                    if key == prev_key:
                        waits = ((inst.get("sync_info") or {})
                                 .get("on_wait") or [])
                        if waits:
                            out.append({
                                "debug": inst.get("debug"),
                                "engine": inst["engine"],
                                "ins": [], "outs": [],
                                "name": f"{inst['name']}.ldwskip",
                                "opcode": "NoOp",
                                "sync_info": inst["sync_info"],
                                "text_hint": "ldw_elided",
                            })
                        continue
                    prev_key = key
                    out.append(inst)
                    continue
                if op not in _LDW_TRANSPARENT:
                    prev_key = None
                out.append(inst)
            blk["instructions"] = out
    return m


def _split_multi_waits(m):
    for f in m.get("functions", []):
        for blk in f.get("blocks", []):
            out = []
            changed = False
            for inst in blk.get("instructions", []):
                si = inst.get("sync_info")
                waits = (si or {}).get("on_wait") or []
                if len(waits) > 1:
                    changed = True
                    for k, wcond in enumerate(waits[:-1]):
                        out.append({
                            "debug": inst.get("debug"),
                            "engine": inst["engine"],
                            "ins": [], "outs": [],
                            "name": f"{inst['name']}.sw{k}",
                            "opcode": "NoOp",
                            "sync_info": {"on_update": [], "on_wait": [wcond]},
                            "text_hint": "split_wait",
                        })
                    si["on_wait"] = [waits[-1]]
                out.append(inst)
            if changed:
                blk["instructions"] = out
    return m


def _to_json_bytes_split(self, *a, **kw):
    data = _orig_to_json_bytes(self, *a, **kw)
    m = _elide_redundant_ldweights(_regroup_matmuls(json.loads(data)))
    return json.dumps(_split_multi_waits(m)).encode()


def _install_patch():
    if bass.Bass.to_json_bytes is not _to_json_bytes_split:
        bass.Bass.to_json_bytes = _to_json_bytes_split


# ---------------------------------------------------------------------------
# Device kernel


def _build_nc():
    _install_patch()
    nc = bass.Bass()
    xin = nc.declare_dram_parameter("xin", [BLOC, PD, PADH, PADW], BF16,
                                    isOutput=False)
    # device layout: k-major so the SBUF load is contiguous per partition
    wts = nc.declare_dram_parameter("wts", [BLOC, 128, NJ * NKX * 128], BF16,
                                    isOutput=False)
    yout = nc.declare_dram_parameter("yout", [BLOC, PD, H, W], F32,
                                     isOutput=True)
    yout_ap = yout.ap()

    with tile.TileContext(nc) as tc:
        with tc.tile_pool(name="wp", bufs=2) as wp, \
             tc.tile_pool(name="xp", bufs=6) as xp, \
             tc.tile_pool(name="pp", bufs=2, space="PSUM") as pp, \
             tc.tile_pool(name="op", bufs=8) as op:

            # input DMAs only on sync+scalar; outputs keep gpsimd to
            # themselves (output DMAs are gated on PSUM drains -- sharing a
            # FIFO queue with them would serialize input prefetch with
            # compute).  Each transfer is one contiguous run per partition
            # so the DMA engines move large bursts instead of row packets.
            qs = [nc.sync, nc.sync, nc.scalar, nc.scalar]

            def band_part(x4, b, band, g, r0, r1, q):
                xin_ap = xin.ap()
                src = bass.AP(
                    xin_ap.tensor,
                    xin_ap.offset
                    + (b * PD * PADH + 32 * band + g + r0) * PADW,
                    [[PADH * PADW, PD], [1, (r1 - r0) * PADW]])
                q.dma_start(x4[32 * g:32 * (g + 1), r0 * PADW:r1 * PADW],
                            src)

            def load_band(b, band):
                x4 = xp.tile([128, SROWS * PADW + 16], BF16, tag="x4")
                for g in range(G):
                    band_part(x4, b, band, g, 0, SROWS, qs[g])
                return x4

            def load_wt(b):
                wt = wp.tile([128, NBLK * 128], BF16, tag="wt")
                src = wts.ap()[b]
                nc.sync.dma_start(wt[:64], src[:64])
                nc.scalar.dma_start(wt[64:], src[64:])
                return wt

            steps = [(b, band) for b in range(BLOC) for band in range(BANDS)]
            # startup: band 0 + the j=0 weight blocks are the critical
            # path; stripe them over all three queues (outputs haven't
            # started) ahead of the remaining weights and rows
            # acc-outer scheduling means the first PSUM pass consumes ALL
            # weight blocks within ~2us of starting, but only band rows
            # <=16 (rows <= 8p+16 for pass p).  Stripe in need order:
            # j=0 weights, rows 0-16, remaining weights, then later rows.
            wt0 = wp.tile([128, NBLK * 128], BF16, tag="wt")
            w0src = wts.ap()[0]
            c0 = NKX * 128   # the j=0 blocks, consumed first
            cm = c0 + (NBLK * 128 - c0) // 2
            nc.sync.dma_start(wt0[:64, :c0], w0src[:64, :c0])
            nc.scalar.dma_start(wt0[64:, :c0], w0src[64:, :c0])
            x4_next = xp.tile([128, SROWS * PADW + 16], BF16, tag="x4")
            q3 = [nc.sync, nc.scalar, nc.gpsimd, nc.gpsimd]
            for g in range(G):
                band_part(x4_next, 0, 0, g, 0, 16, q3[g])
            nc.sync.dma_start(wt0[:64, c0:cm], w0src[:64, c0:cm])
            nc.scalar.dma_start(wt0[64:, c0:cm], w0src[64:, c0:cm])
            nc.gpsimd.dma_start(wt0[:64, cm:], w0src[:64, cm:])
            nc.gpsimd.dma_start(wt0[64:, cm:], w0src[64:, cm:])
            for g in range(G):
                band_part(x4_next, 0, 0, g, 16, 32, q3[g])
            for g in range(G):
                band_part(x4_next, 0, 0, g, 32, SROWS,
                          [nc.sync, nc.scalar, nc.sync, nc.scalar][g])
            # wt1 is not needed until step 6; defer it so early band
            # prefetch isn't queued behind its 1.7MB
            wtiles = [wt0, None]
            for si, (b, band) in enumerate(steps):
                wt = wtiles[b]
                y0 = 32 * band
                x4 = x4_next
                if si + 1 < len(steps):
                    x4_next = load_band(*steps[si + 1])
                if si == 2:
                    wtiles[1] = load_wt(1)
                x4a = x4[:]
                # weight-block-outer order: each block feeds all 4 pairs
                # back-to-back so walrus's redundant-LDWEIGHTS elision
                # (--enable-ldw-opt) drops 3 of every 4 weight loads
                accs = [pp.tile([128, NFREE], F32, tag=f"acc{p}",
                                name=f"acc{p}_{si}")
                        for p in range(PAIRS)]
                for j in range(NJ):
                    for kx in range(NKX):
                        wblk = wt[:, (j * NKX + kx) * 128:
                                     (j * NKX + kx + 1) * 128]
                        for p in range(PAIRS):
                            s0 = 8 * p + 4 * j
                            rhs = bass.AP(
                                x4a.tensor,
                                x4a.offset + s0 * PADW + kx,
                                [list(x4a.ap[0]),
                                 [4 * PADW, 2], [1, W]])
                            nc.tensor.matmul(
                                accs[p][:], wblk, rhs,
                                start=(j == 0 and kx == 0),
                                stop=(j == NJ - 1 and kx == NKX - 1))
                last = si == len(steps) - 1
                for p in range(PAIRS):
                    ot = op.tile([128, NFREE], F32, tag="ot")
                    # drain each PSUM bank with two engines in parallel
                    nc.vector.tensor_copy(ot[:, :W], accs[p][:, :W])
                    nc.scalar.copy(ot[:, W:], accs[p][:, W:])
                    for q in range(2):
                        src = ot[:, q * W:q * W + W]
                        dst = bass.AP(
                            yout_ap.tensor,
                            b * PD * H * W + (y0 + 8 * p + 4 * q) * W,
                            [[W, DY], [H * W, PD], [1, W]])
                        # input queues are idle by the last step; use them
                        # to shorten the output tail
                        oq = ([nc.gpsimd, nc.sync, nc.scalar][
                            (2 * p + q) % 3] if last else nc.gpsimd)
                        oq.dma_start(dst, src)
    return nc


_NC = None


def _get_nc():
    global _NC
    if _NC is None:
        _NC = _build_nc()
    return _NC


# ---------------------------------------------------------------------------
# Host side


def _gelu_exact(z):
    from math import erf
    return 0.5 * z * (1.0 + np.vectorize(erf)(z / np.sqrt(2.0)))


def _prepare_inputs(x, lk_filter, w1, b1, w2, b2):
    x = np.ascontiguousarray(np.asarray(x, dtype=np.float32))
    x1 = x[:, :PD]

    # dwc_proj on host (tiny): pool -> 1x1 -> exact GELU -> 1x1
    pooled = x1.mean(axis=(2, 3), dtype=np.float32)            # [B, 32]
    hid = _gelu_exact(pooled @ np.asarray(w1, np.float32).T
                      + np.asarray(b1, np.float32)).astype(np.float32)
    dyn_k = (hid @ np.asarray(w2, np.float32).T
             + np.asarray(b2, np.float32)).reshape(B, PD, SK, SK)

    # fold the per-sample depthwise 3x3 into the center of the 13x13 filter
    F = np.broadcast_to(np.asarray(lk_filter, np.float32),
                        (B, PD, PD, LK, LK)).copy()
    idx = np.arange(PD)
    ctr = PAD - SK // 2                                         # 5
    F[:, idx, idx, ctr:ctr + SK, ctr:ctr + SK] += dyn_k

    # weight blocks: wts[b, j, kx, g*32+ic, dy*32+oc] = F[b, oc, ic, 4j+g-dy, kx]
    import ml_dtypes
    wts = np.zeros((B, NJ, NKX, 128, 128), np.float32)
    for j in range(NJ):
        for g in range(G):
            for dy in range(DY):
                ky = 4 * j + g - dy
                if 0 <= ky < LK:
                    wts[:, j, :, g * PD:(g + 1) * PD,
                        dy * PD:(dy + 1) * PD] = \
                        F[:, :, :, ky, :].transpose(0, 3, 2, 1)
    # device layout: [b, k, (j, kx, m)] so the SBUF load is contiguous
    wts = np.ascontiguousarray(wts.transpose(0, 3, 1, 2, 4)).reshape(
        B, 128, NJ * NKX * 128).astype(ml_dtypes.bfloat16)

    xpad = np.zeros((B, PD, PADH, PADW), ml_dtypes.bfloat16)
    xpad[:, :, PAD:PAD + H, PAD:PAD + W] = x1

    in_maps = [{"xin": xpad[BLOC * c:BLOC * (c + 1)],
                "wts": wts[BLOC * c:BLOC * (c + 1)]}
               for c in range(NCORES)]
    return x, in_maps


def _execute(in_maps, trace=False):
    nc = _get_nc()
    return run_bass_kernel_spmd(nc, in_maps, list(range(NCORES)), trace=trace)


def kernel(x, lk_filter, w1, b1, w2, b2):
    x, in_maps = _prepare_inputs(x, lk_filter, w1, b1, w2, b2)
    res = _execute(in_maps)
    out = np.empty((B, C, H, W), np.float32)
    for c in range(NCORES):
        out[BLOC * c:BLOC * (c + 1), :PD] = res.results[c]["yout"]
    out[:, PD:] = x[:, PD:]
    return out

